# revision 4
# baseline (speedup 1.0000x reference)
"""Trainium2 Bass kernel for CombinedKSpaceRowwiseMSELoss.

loss = sum over channels of [ mean((pred-target)^2 over central cols)
                              + mean(|pred-target| over periphery cols) ]

Strategy
--------
Pure data parallel over the batch dim: 32 batches / 8 cores = 4 per core.
The whole problem is memory-bound (the only real cost is streaming every
element of pred and target from HBM once), so the kernel is built around
the DMA stream:

- The host shards per core and packs pred+target *interleaved per
  (tile, partition)* and cast to bf16: each SBUF partition line is
  [pred 2*640 | tgt 2*640] from one contiguous DRAM chunk, so each tile is
  a single DMA with 128 contiguous descriptors. bf16 halves the HBM bytes
  (the sole bottleneck); the quantization error on this loss is ~1e-6
  relative (tolerance is 2e-2) since the on-device sums run in fp32.
- 20 tiles of 2 rows/partition, io_pool 8 deep, ALL input DMAs issued from
  the SP HWDGE ring only. The ACT ring never carries input DMAs, so DMA
  descriptor posting never waits behind compute in an engine's in-order
  instruction stream (this interleaving cost ~14us/pass when the ACT ring
  carried half the input stream).
- Per tile: DVE computes diff = pred - target (bf16, 2x mode); ACT squares
  the 160 central cols with a fused fp32 row-accumulate (accum_out); DVE
  abs-sum-reduces the two periphery slices with fused |x| into fp32.
  Compute (~25us) hides completely under the ~37us DMA stream.
- One fp32 accumulator tile [128, 3T] -> ONE output DMA, issued from the
  ACT ring for the same never-block-the-input-ring reason.
- Final ~7.7K-element reduction + normalization happen on the host in f64.
"""

import sys

for _p in ("/opt/trn_rl_repo",):
    if _p not in sys.path:
        sys.path.insert(0, _p)

import numpy as np
from contextlib import ExitStack

import concourse.bass as bass
import concourse.tile as tile
from concourse import bacc, mybir
from concourse.bass_utils import run_bass_kernel_spmd

N_CORES = 8
B, C, H, W = 32, 2, 640, 640
B_SHARD = B // N_CORES          # 4 batch elements per core
ROWS = B_SHARD * C * H          # 5120 rows per core per tensor
P = 128                         # SBUF partitions
RPT = ROWS // P                 # 40 rows per partition total
CW = int(W * 0.25)              # 160 central cols
CS = (W - CW) // 2              # 240
CE = CS + CW                    # 400
PW = W - CW                     # 480 periphery cols per row

F32 = mybir.dt.float32
_DT = {"f32": mybir.dt.float32, "bf16": mybir.dt.bfloat16}

TILE_PLAN = (2,) * 20           # rows/partition per tile
assert sum(TILE_PLAN) == RPT

IN_DTYPE = "bf16"
IO_BUFS = 8
RINGS = "sp"


def build_program(
    loop_n: int = 1,
    mode: str = "full",
    plan: tuple = TILE_PLAN,
    io_bufs: int = IO_BUFS,
    rings: str = RINGS,
    in_dtype: str = IN_DTYPE,
    act_split: bool = True,
) -> bass.Bass:
    assert sum(plan) == RPT
    T = len(plan)
    DT = _DT[in_dtype]
    nc = bacc.Bacc("TRN2", target_bir_lowering=False, debug=False)

    pt = nc.dram_tensor("pt", [2 * ROWS, W], DT, kind="ExternalInput")
    acc_out = nc.dram_tensor("acc", [P, 3 * T], F32, kind="ExternalOutput")

    with tile.TileContext(nc) as tc:
        with ExitStack() as ctx:
            io_pool = ctx.enter_context(tc.tile_pool(name="io", bufs=io_bufs))
            work_pool = ctx.enter_context(tc.tile_pool(name="work", bufs=3))
            acc_pool = ctx.enter_context(tc.tile_pool(name="acc", bufs=1))

            # acc[:, :T] central sq-sums, acc[:, T:3T] periphery abs-sums
            acc = acc_pool.tile([P, 3 * T], F32)
            if mode == "dma":
                nc.vector.memset(acc[:], 0.0)

            def emit_tile(t, rpp, row0):
                rw = rpp * W
                io = io_pool.tile([P, 2 * rw], DT, tag=f"io{rpp}")
                src = pt.ap()[2 * row0 : 2 * (row0 + P * rpp)].rearrange(
                    "(p s r) w -> p (s r w)", p=P, s=2, r=rpp
                )
                if rings == "sp":
                    eng = nc.sync
                elif rings == "act":
                    eng = nc.scalar
                else:  # alternate
                    eng = nc.sync if t % 2 == 0 else nc.scalar
                eng.dma_start(io[:], src)
                if mode == "dma":
                    return

                diff = work_pool.tile([P, rw], DT, tag=f"diff{rpp}")
                nc.vector.tensor_sub(diff[:], io[:, :rw], io[:, rw:])
                d3 = diff[:].rearrange("p (r w) -> p r w", w=W)

                # central: fused square + fp32 row-accumulate on ACT
                sq = work_pool.tile([P, rpp * CW], DT, tag=f"sq{rpp}")
                nc.scalar.activation(
                    sq[:].rearrange("p (r w) -> p r w", w=CW),
                    d3[:, :, CS:CE],
                    mybir.ActivationFunctionType.Square,
                    accum_out=acc[:, t : t + 1],
                )
                # periphery: fused |x| + fp32 row-accumulate, one slice per
                # engine so neither DVE nor ACT exceeds the DMA stream time
                # (Abs is a filler function in every ACT table set, so it
                # shares a set with Square -> no table-swap cost).
                if act_split:
                    ab = work_pool.tile([P, rpp * CS], DT, tag=f"ab{rpp}")
                    nc.scalar.activation(
                        ab[:].rearrange("p (r w) -> p r w", w=CS),
                        d3[:, :, 0:CS],
                        mybir.ActivationFunctionType.Abs,
                        accum_out=acc[:, T + 2 * t : T + 2 * t + 1],
                    )
                else:
                    nc.vector.tensor_reduce(
                        acc[:, T + 2 * t : T + 2 * t + 1],
                        d3[:, :, 0:CS],
                        axis=mybir.AxisListType.XY,
                        op=mybir.AluOpType.add,
                        apply_absolute_value=True,
                    )
                nc.vector.tensor_reduce(
                    acc[:, T + 2 * t + 1 : T + 2 * t + 2],
                    d3[:, :, CE:],
                    axis=mybir.AxisListType.XY,
                    op=mybir.AluOpType.add,
                    apply_absolute_value=True,
                )

            def body():
                row0 = 0
                for t, rpp in enumerate(plan):
                    emit_tile(t, rpp, row0)
                    row0 += P * rpp
                # Output DMA on the ACT ring so it never blocks the SP ring
                # that streams the input tiles (HWDGE rings are FIFO per
                # issuing engine).
                nc.scalar.dma_start(acc_out.ap(), acc[:])

            if loop_n > 1:
                with tc.For_i(0, loop_n, 1):
                    body()
            else:
                body()

    nc.compile()
    return nc


_CACHED_NC = None


def _get_program() -> bass.Bass:
    global _CACHED_NC
    if _CACHED_NC is None:
        _CACHED_NC = build_program()
    return _CACHED_NC


def _np_dtype(in_dtype: str):
    if in_dtype == "bf16":
        import ml_dtypes

        return ml_dtypes.bfloat16
    return np.float32


def _pack_core(predc, tgtc, plan, npdt):
    """predc/tgtc: [ROWS, W] for one core -> packed [2*ROWS, W]."""
    out = np.empty((2 * ROWS, W), npdt)
    row0 = 0
    for rpp in plan:
        n = P * rpp
        blk = out[2 * row0 : 2 * (row0 + n)].reshape(P, 2, rpp, W)
        blk[:, 0] = predc[row0 : row0 + n].reshape(P, rpp, W)
        blk[:, 1] = tgtc[row0 : row0 + n].reshape(P, rpp, W)
        row0 += n
    return out


def shard_inputs(
    pred: np.ndarray,
    target: np.ndarray,
    plan: tuple = TILE_PLAN,
    in_dtype: str = IN_DTYPE,
):
    npdt = _np_dtype(in_dtype)
    pred = np.asarray(pred).astype(npdt, copy=False).reshape(B, C * H, W)
    target = np.asarray(target).astype(npdt, copy=False).reshape(B, C * H, W)
    in_maps = []
    for i in range(N_CORES):
        sl = slice(i * B_SHARD, (i + 1) * B_SHARD)
        predc = pred[sl].reshape(ROWS, W)
        tgtc = target[sl].reshape(ROWS, W)
        in_maps.append({"pt": _pack_core(predc, tgtc, plan, npdt)})
    return in_maps


def reduce_partials(results: list, plan: tuple = TILE_PLAN) -> np.ndarray:
    T = len(plan)
    tot_sq = 0.0
    tot_abs = 0.0
    for r in results:
        a = r["acc"].astype(np.float64)
        tot_sq += a[:, :T].sum()
        tot_abs += a[:, T:].sum()
    loss = tot_sq / (B * H * CW) + tot_abs / (B * H * PW)
    return np.asarray(loss, dtype=np.float32)


def kernel(pred: np.ndarray, target: np.ndarray) -> np.ndarray:
    nc = _get_program()
    in_maps = shard_inputs(pred, target)
    res = run_bass_kernel_spmd(nc, in_maps, list(range(N_CORES)))
    return reduce_partials(res.results)
